# revision 66
# baseline (speedup 1.0000x reference)
"""Trainium2 Bass kernel for nn_NormalizedDelinear (whitened linear layer).

Math (reference):
    X = x.reshape(-1, 512); N = X.shape[0]
    mean = X.mean(0);  cov = eps*I + (X-mean)^T (X-mean) / N
    C = newton_schulz_isqrt(cov, 5)
    w = weight.reshape(-1, 512) @ C;  b = bias - (w @ mean).reshape(1024, 2).sum(1)
    out = x @ w.reshape(1024, 1024).T + b

Distribution: data-parallel over the 65536 rows of x across 8 NeuronCores.
Each core computes partial S = X_loc^T X_loc and column sums s, a single
~1 MB AllReduce combines them, every core runs the (cheap, replicated)
Newton-Schulz and weight transform, then computes its slice of the output
GEMM.

Pipeline per core:
  pass A: stream x in f32 chunks via HWDGE (full HBM rate), cast f32->bf16
          on the ACT engine, accumulate S = X^T X (upper triangle) on the
          PE plus column sums on the DVE, and write the bf16 sample-major
          copy back to DRAM (xbf).
  AR:     AllReduce S + column-sum partials.
  NS:     replicated Newton-Schulz in bf16 (fp32 PSUM), then the weight
          transform wT = blockdiag(C,C) @ W^T and bias correction.
  pass D: per 128-row tile, load x^T tiles from xbf with the DMA xbar
          transpose (contiguous DRAM reads), run the 16 N=512 matmuls
          back-to-back, add bias on DVE, write out.
"""
import numpy as np

import concourse.bacc as bacc
import concourse.mybir as mybir
import concourse.tile as tile
import concourse.bass_utils as bass_utils

N_CORES = 8
ROWS = 65536
D = 1024
BLOCK = 512
EPS = 1e-5
N_ITER = 5
PART = 128
ROWS_PER_CORE = ROWS // N_CORES  # 8192
N_ROW_TILES = ROWS_PER_CORE // PART  # 64
TILES_PER_CHUNK = 2

f32 = mybir.dt.float32
bf16 = mybir.dt.bfloat16
ADD = mybir.AluOpType.add
MUL = mybir.AluOpType.mult


def build_nc(n_row_tiles=N_ROW_TILES):
    nc = bacc.Bacc(
        "TRN2", target_bir_lowering=False, debug=False, num_devices=N_CORES
    )
    rows_pc = n_row_tiles * PART
    n_chunks = max(1, n_row_tiles // TILES_PER_CHUNK)
    tpc = n_row_tiles // n_chunks
    n_total = rows_pc * N_CORES * (D // BLOCK)  # global sample count N

    x = nc.dram_tensor("x", [rows_pc, D], f32, kind="ExternalInput")
    weight = nc.dram_tensor("weight", [D, D], f32, kind="ExternalInput")
    bias_rep = nc.dram_tensor("bias_rep", [PART, D], f32, kind="ExternalInput")
    # single diagonal block: eye15[p, f] = 1.5 * (p == f)
    eye15 = nc.dram_tensor("eye15", [PART, PART], bf16, kind="ExternalInput")
    id_bf16 = nc.dram_tensor("id_bf16", [PART, PART], bf16, kind="ExternalInput")
    id_f32 = nc.dram_tensor("id_f32", [PART, PART], f32, kind="ExternalInput")
    out = nc.dram_tensor("out", [rows_pc, D], f32, kind="ExternalOutput")

    with tile.TileContext(nc) as tc:
        _kernel_body(
            nc, tc, x, weight, bias_rep, eye15, id_bf16, id_f32, out,
            n_row_tiles, n_chunks, tpc, n_total,
        )
    nc.compile()
    return nc


def _kernel_body(
    nc, tc, x, weight, bias_rep, eye15, id_bf16, id_f32, out,
    n_row_tiles, n_chunks, tpc, n_total,
):
    inv_n = 1.0 / float(n_total)

    # ------------- long-lived pools (left side) + DRAM -------------------
    consts = tc.alloc_tile_pool(name="consts", bufs=1, side="left")
    dram = tc.alloc_tile_pool(name="dram", bufs=1, space="DRAM")

    eye15_sb = consts.tile([PART, PART], bf16, tag="eye15")
    id_bf = consts.tile([PART, PART], bf16, tag="id_bf")
    id_f = consts.tile([PART, PART], f32, tag="id_f")
    ones_f = consts.tile([PART, 1], f32, tag="ones_f")
    ones_row = consts.tile([1, PART], f32, tag="ones_row")
    ones_bf = consts.tile([PART, PART], bf16, tag="ones_bf")

    nc.sync.dma_start(eye15_sb[:], eye15[:])
    nc.sync.dma_start(id_bf[:], id_bf16[:])
    nc.sync.dma_start(id_f[:], id_f32[:])
    nc.vector.memset(ones_f[:], 1.0)
    nc.vector.memset(ones_row[:], 1.0)
    nc.vector.memset(ones_bf[:], 1.0)

    # bf16 sample-major copy of x, written back during pass A and read with
    # the DMA xbar transpose in pass D.
    xbf = dram.tile([n_row_tiles * PART, D], bf16, tag="xbf")

    # long-lived left-side pools, allocated in lifetime order (LIFO stack):
    # wts2 (wT, lives through pass D), pa (acc), wth (WThs/C/rep, until the
    # weight transform), early (S/A scratch, until Y0).
    wts2 = tc.alloc_tile_pool(name="wts2", bufs=1, side="left")
    pa = tc.alloc_tile_pool(name="passA", bufs=1, side="left")

    wth = tc.alloc_tile_pool(name="wth", bufs=1, side="left")

    b_rep = consts.tile([PART, D], f32, tag="b_rep")  # b' replicated later
    nc.sync.dma_start(b_rep[:], bias_rep[:])

    # ------------- pass A: stream x, cast, S = X^T X, col sums, writeback -
    acc = pa.tile([PART, BLOCK], f32, tag="acc")  # running column sums
    nc.vector.memset(acc[:], 0.0)

    # bf16 weight (sample-major) staged in SBUF; its 128x128 blocks are
    # PE-transposed into WThs during pass A's PE slack.  This avoids both
    # the DRAM->DRAM cast DMA (which monopolized all 16 DMA engines for
    # ~30us at startup) and the W^T xbar-transpose reads (256B packets).
    W_bf = wth.tile([PART, 8, D], bf16, tag="W_bf")
    WThs = [
        wth.tile([PART, 4, D], bf16, tag=f"WTh{j}", name=f"WTh{j}")
        for j in range(2)
    ]

    # stageB2 holds the LAST half of the bf16 chunks so their xbf writeback
    # can be deferred into the AllReduce/Newton-Schulz bubble, cutting pass
    # A's HBM traffic from 48 MB to 40 MB.  One tile, ONE writeback DMA
    # (16 separate triggers cost ~20us of scalar-queue time).  Allocated
    # first: it outlives the rotating stage pools (LIFO stack).
    stageB2 = tc.alloc_tile_pool(name="stageB2", bufs=1, side="right")
    wstage = tc.alloc_tile_pool(name="wstage", bufs=2, side="right")
    stageF = tc.alloc_tile_pool(name="stageF", bufs=5, side="right")
    stageB = tc.alloc_tile_pool(name="stageB", bufs=5, side="right")

    # ps_wt below ps_S on the right-PSUM stack: ps_S releases first (at the
    # AR pack) while ps_wt lives on for the weight transposes in the bubble.
    ps_wt = tc.alloc_tile_pool(name="psumWT", bufs=2, space="PSUM", side="right")
    ps_S = tc.alloc_tile_pool(name="psumS", bufs=1, space="PSUM", side="right")
    # upper-triangle blocks of S: block row m covers columns [m*128, 512)
    s_psum = [
        ps_S.tile([PART, BLOCK - m * PART], f32, tag=f"S{m}", name=f"S{m}")
        for m in range(4)
    ]

    def w_transpose_all():
        # WTh[j][p, db, ob*128+n] = weight[ob*128+n, j*512+db*128+p].
        # One transpose per PSUM tile: transpose-mode writes must start at
        # a bank boundary (offset writes fault the exec unit).
        for j in range(2):
            for db in range(4):
                for ob in range(8):
                    tp = ps_wt.tile([PART, PART], bf16, tag="wtp", name="wtp")
                    src = (j * 4 + db) * PART
                    nc.tensor.transpose(
                        tp[:], W_bf[:, ob, src:src + PART], id_bf[:]
                    )
                    nc.vector.tensor_copy(
                        WThs[j][:, db, ob * PART:(ob + 1) * PART], tp[:]
                    )

    first = True
    N_DEF = min(16, n_chunks // 2)
    c_def0 = n_chunks - N_DEF
    sb2_all = stageB2.tile([PART, N_DEF * tpc, D], bf16, tag="sb2")
    for c in range(n_chunks):
        rows = slice(c * tpc * PART, (c + 1) * tpc * PART)
        sf = stageF.tile([PART, tpc, D], f32, tag="sf", name="sf")
        if c == 0:
            # per-tile loads for the first chunk so the first cast (and the
            # first matmul) only waits on 0.5 MB instead of the whole chunk
            for t in range(tpc):
                r0 = c * tpc * PART + t * PART
                nc.sync.dma_start(
                    sf[:, t, :],
                    x[r0:r0 + PART, :].rearrange("(t p) f -> p (t f)", p=PART),
                )
        else:
            nc.sync.dma_start(
                sf[:], x[rows, :].rearrange("(t p) f -> p t f", p=PART)
            )
        if c == 0:
            # weight f32 load; bf16 cast on DVE
            for q in range(4):
                wf = wstage.tile([PART, 2, D], f32, tag="wf", name="wf")
                nc.sync.dma_start(
                    wf[:],
                    weight[q * 2 * PART:(q + 1) * 2 * PART, :].rearrange(
                        "(t p) f -> p t f", p=PART
                    ),
                )
                nc.vector.tensor_copy(W_bf[:, q * 2:(q + 1) * 2, :], wf[:])
        defer = c >= c_def0
        if defer:
            sb = sb2_all[:, (c - c_def0) * tpc:(c - c_def0 + 1) * tpc, :]
        else:
            sb = stageB.tile([PART, tpc, D], bf16, tag="sb", name="sb")
        for t in range(tpc):
            nc.scalar.copy(sb[:, t, :], sf[:, t, :])  # ACT f32 -> bf16 cast
        # writeback of the bf16 sample-major rows (read again in pass D);
        # on the scalar queue (same engine as the casts) so its semaphore
        # wait clears in-order and never stalls another queue's triggers.
        # The last N_DEF chunks write back in one DMA during the AR bubble.
        if not defer:
            nc.scalar.dma_start(
                xbf[rows, :].rearrange("(t p) f -> p t f", p=PART), sb[:]
            )
        for t in range(tpc):
            for h in range(2):
                xt = sb[:, t, h * BLOCK:(h + 1) * BLOCK]  # [128, 512] bf16
                for m in range(4):
                    nc.tensor.matmul(
                        s_psum[m][:],
                        xt[:, m * PART:(m + 1) * PART],
                        xt[:, m * PART:],
                        start=first,
                        stop=(c == n_chunks - 1 and t == tpc - 1 and h == 1),
                    )
                # column-sum accumulator on DVE (f32 += bf16)
                nc.vector.tensor_add(acc[:], acc[:], xt)
                first = False

    stageB.release()
    stageF.release()
    wstage.release()

    # pass-D SBUF pools allocated early so the first x^T tiles can prefetch
    # during the AllReduce + Newton-Schulz window.
    pd = tc.alloc_tile_pool(name="passD", bufs=5, side="right")
    pd_out = tc.alloc_tile_pool(name="passDout", bufs=5, side="right")
    N_PRE_PAIRS = 4

    def xT_load(pi):
        # x^T tiles for row-block pair pi (two 128-row tiles per transpose):
        # xT8[i, g, n] = x[pi*256+n, g*128+i], n in [0, 256).  All transposes
        # stay on the scalar HWDGE queue.
        xT8 = pd.tile([PART, 8, 2 * PART], bf16, tag="xT", name="xT")
        nc.scalar.dma_start(
            xT8[:], xbf[pi * 2 * PART:(pi + 1) * 2 * PART, :], transpose=True
        )
        return xT8

    xT_pre = None  # issued after the AllReduce trigger (see below)

    # ------------- pack upper-triangle S + col-sum partials, AllReduce ----
    early = tc.alloc_tile_pool(name="early", bufs=1, side="left")
    late = tc.alloc_tile_pool(name="late", bufs=1, side="right")
    ps_asm = tc.alloc_tile_pool(name="psumA", bufs=2, space="PSUM", side="left")

    # compact upper-triangle staging: block row m (width 512-m*128) at TRI_OFF[m]
    TRI_OFF = [0, 512, 896, 1152]
    TRI_W = 1280
    S_tri = early.tile([PART, TRI_W], f32, tag="S_tri")
    for m in range(4):
        nc.vector.tensor_copy(
            S_tri[:, TRI_OFF[m]:TRI_OFF[m] + BLOCK - m * PART], s_psum[m][:]
        )
    ps_S.release()

    # AllReduce buffer: [:, 0:1280] = S upper triangle, [:, 1280:1792] =
    # per-partition column-sum partials (896 KB instead of 1.25 MB).
    AR_W = TRI_W + BLOCK
    ar_in = dram.tile([PART, AR_W], f32, tag="ar_in")
    ar_out = dram.tile([PART, AR_W], f32, tag="ar_out", addr_space="Shared")
    nc.sync.dma_start(ar_in[:, 0:TRI_W], S_tri[:])
    nc.sync.dma_start(ar_in[:, TRI_W:], acc[:])
    nc.gpsimd.collective_compute(
        "AllReduce",
        ADD,
        replica_groups=[list(range(N_CORES))],
        ins=[ar_in.opt()],
        outs=[ar_out.opt()],
    )

    # Bubble fillers, issued AFTER the pack + AR trigger so they can't
    # delay it: the 64 weight transposes run on the idle PE/DVE during the
    # AllReduce, and the first pass-D x^T prefetches run on the idle DMA.
    w_transpose_all()
    ps_wt.release()
    def_rows = slice(c_def0 * tpc * PART, n_chunks * tpc * PART)
    nc.scalar.dma_start(
        xbf[def_rows, :].rearrange("(t p) f -> p t f", p=PART), sb2_all[:]
    )
    xT_pre = [xT_load(pi) for pi in range(N_PRE_PAIRS)]

    # ------------- unpack AllReduce, build A = cov -----------------------
    S_sb = early.tile([PART, 4, BLOCK], f32, tag="S_sb")  # also reused as A
    for m in range(4):
        nc.sync.dma_start(
            S_sb[:, m, m * PART:], ar_out[:, TRI_OFF[m]:TRI_OFF[m] + BLOCK - m * PART]
        )
    nc.sync.dma_start(acc[:], ar_out[:, TRI_W:])  # now global partials
    # lower triangle: block (m, b) with b < m = transpose of block (b, m)
    for m in range(4):
        for b in range(m):
            tp = ps_asm.tile([PART, BLOCK], f32, tag="t", name="tp")
            nc.tensor.transpose(
                tp[:, 0:PART], S_sb[:, b, m * PART:(m + 1) * PART], id_f[:]
            )
            nc.vector.tensor_copy(S_sb[:, m, b * PART:(b + 1) * PART], tp[:, 0:PART])

    # global column sums s = ones^T @ acc -> [1, 512]
    scol = ps_asm.tile([PART, BLOCK], f32, tag="t")
    nc.tensor.matmul(scol[0:1, :], ones_f[:], acc[:])
    s_sb = late.tile([1, BLOCK], f32, tag="s_sb")
    nc.vector.tensor_copy(s_sb[:], scol[0:1, :])

    # meanrowN[p, c] = N*mean[c] (replicated down partitions), via PE ones
    mr_ps = ps_asm.tile([PART, BLOCK], f32, tag="t")
    nc.tensor.matmul(mr_ps[:], ones_row[:], s_sb[:])
    meanrowN = early.tile([PART, BLOCK], f32, tag="meanrowN")
    nc.vector.tensor_copy(meanrowN[:], mr_ps[:])

    # mean_sb[p, b] = mean[b*128+p], via PE transposes of meanrowN blocks
    mean_sb = late.tile([PART, 4], f32, tag="mean_sb")
    for b in range(4):
        mt = ps_asm.tile([PART, BLOCK], f32, tag="t", name="mt")
        nc.tensor.transpose(
            mt[:, 0:PART], meanrowN[:, b * PART:(b + 1) * PART], id_f[:]
        )
        nc.vector.tensor_scalar_mul(mean_sb[:, b:b + 1], mt[:, 0:1], inv_n)

    # Work with the unscaled A'' = N*cov = S - (N mean) mean^T + N*eps*I;
    # the 1/N folds into the Frobenius-norm scalars below.
    A = S_sb  # A'' built in place over the all-reduced S
    scratch = early.tile([PART, BLOCK], f32, tag="scratch")
    eps_st = early.tile([PART, PART], f32, tag="eps_st")
    nc.scalar.mul(eps_st[:], eye15_sb[:], float(n_total) * EPS / 1.5)
    # One fused DVE op per block builds the NEGATED A'': (N mean)mean^T - S;
    # the sign is folded into the Y0 scale below (Frobenius is sign-blind).
    SUB = mybir.AluOpType.subtract
    for b in range(4):
        nc.vector.scalar_tensor_tensor(
            A[:, b, :], meanrowN[:], mean_sb[:, b:b + 1], A[:, b, :],
            op0=MUL, op1=SUB,
        )
        # - N*eps*I on the diagonal 128-wide stripe of this block row
        d0 = b * PART
        nc.vector.tensor_sub(
            A[:, b, d0:d0 + PART], A[:, b, d0:d0 + PART], eps_st[:]
        )

    # ------------- Frobenius norm of A''; fold N back into r, q ----------
    rowsq4 = early.tile([PART, 4], f32, tag="rowsq4")
    for b in range(4):
        nc.vector.tensor_mul(scratch[:], A[:, b, :], A[:, b, :])
        nc.vector.tensor_reduce(
            rowsq4[:, b:b + 1], scratch[:], mybir.AxisListType.X, ADD
        )
    rowsq = early.tile([PART, 1], f32, tag="rowsq")
    nc.vector.tensor_reduce(rowsq[:], rowsq4[:], mybir.AxisListType.X, ADD)
    n2_ps = ps_asm.tile([PART, BLOCK], f32, tag="t")
    nc.tensor.matmul(n2_ps[0:1, 0:1], ones_f[:], rowsq[:])
    n2_sb = early.tile([1, 1], f32, tag="n2sb")
    nc.vector.tensor_copy(n2_sb[:], n2_ps[0:1, 0:1])
    # broadcast ||A''||^2 to [128, 1] then compute per-partition scalars
    n2_bc = ps_asm.tile([PART, BLOCK], f32, tag="t")
    nc.tensor.matmul(n2_bc[:, 0:1], ones_row[:], n2_sb[:])
    rq = late.tile([PART, 2], f32, tag="rq")
    nc.vector.reciprocal(rq[:, 0:1], n2_bc[:, 0:1])    # 1/||A''||^2
    nc.scalar.sqrt(rq[:, 0:1], rq[:, 0:1])             # r'' = 1/||A''||
    # q = 1/sqrt(||cov||) = sqrt(N) * sqrt(r'')  since ||cov|| = ||A''||/N;
    # Y0 = cov/||cov|| = A''*r'' needs no extra scaling.
    nc.scalar.sqrt(rq[:, 1:2], rq[:, 0:1])
    nc.vector.tensor_scalar_mul(rq[:, 1:2], rq[:, 1:2], float(np.sqrt(n_total)))

    ps_asm.release()

    # ------------- Newton-Schulz (bf16 matmuls, fp32 PSUM) ----------------
    ns = tc.alloc_tile_pool(name="ns", bufs=1, side="right")
    ps_ns = tc.alloc_tile_pool(name="psumNS", bufs=8, space="PSUM", side="left")

    # ping-pong buffers for Y and Z across iterations
    Yb = [ns.tile([PART, 4, BLOCK], bf16, tag=f"Y{i}", name=f"Y{i}") for i in range(2)]
    Zb = [ns.tile([PART, 4, BLOCK], bf16, tag=f"Z{i}", name=f"Z{i}") for i in range(2)]
    T = ns.tile([PART, 4, BLOCK], bf16, tag="T")
    C = wth.tile([PART, 4, BLOCK], bf16, tag="C")
    rep = [
        wth.tile([PART, PART], bf16, tag=f"rep{b}", name=f"rep{b}")
        for b in range(4)
    ]

    for b in range(4):  # Y0 = A''/||A''|| = (stored -A'') * r'' * -1
        nc.vector.tensor_scalar(
            Yb[0][:, b, :], A[:, b, :], rq[:, 0:1], -1.0, op0=MUL, op1=MUL
        )

    early.release()

    def mm512(dst, L, R, copy_engine, scale=None):
        """dst = L(stored)^T @ R for 512x512 bf16 operands laid [128, 4, 512].

        Valid when L is symmetric (or its transpose is wanted). dst must not
        alias L or R. copy_engine: 'v' DVE / 's' ACT for the psum->sbuf copy.
        """
        for mb in range(4):
            pt = ps_ns.tile([PART, BLOCK], f32, tag="mm", name="mm")
            for kb in range(4):
                nc.tensor.matmul(
                    pt[:],
                    L[:, kb, mb * PART:(mb + 1) * PART],
                    R[:, kb, :],
                    start=(kb == 0),
                    stop=(kb == 3),
                )
            if scale is not None:
                nc.vector.tensor_scalar(dst[:, mb, :], pt[:], scale, None, op0=MUL)
            elif copy_engine == "v":
                nc.vector.tensor_copy(dst[:, mb, :], pt[:])
            else:
                nc.scalar.copy(dst[:, mb, :], pt[:])

    def build_T(p_blocks):
        # T = 1.5 I - 0.5 P: full-width scale on ACT, then the diagonal
        # 128-wide 1.5*I add on DVE (eye15 block b is zero off that stripe).
        for b in range(4):
            nc.scalar.mul(T[:, b, :], p_blocks[b], -0.5)
            d0 = b * PART
            nc.vector.tensor_add(
                T[:, b, d0:d0 + PART], T[:, b, d0:d0 + PART], eye15_sb[:]
            )

    # iter 1: Z0 = I, so P = Y0; T1 = 1.5I - 0.5 Y0; Y1 = Y0 @ T1; Z1 = T1
    Y, Z = Yb[0], Zb[0]
    build_T([Y[:, b, :] for b in range(4)])
    mm512(Yb[1], Y, T, "s")  # Y1 = Y0 @ T1  (Y0 symmetric)
    for b in range(4):
        nc.scalar.copy(Zb[1][:, b, :], T[:, b, :])
    Y, Z = Yb[1], Zb[1]

    for it in range(1, N_ITER):
        # P = Z @ Y -> psum tiles; T = 1.5I - 0.5P
        pt_blocks = []
        for mb in range(4):
            pt = ps_ns.tile([PART, BLOCK], f32, tag="mm", name="mm")
            for kb in range(4):
                nc.tensor.matmul(
                    pt[:],
                    Z[:, kb, mb * PART:(mb + 1) * PART],
                    Y[:, kb, :],
                    start=(kb == 0),
                    stop=(kb == 3),
                )
            pt_blocks.append(pt)
        build_T([pt[:] for pt in pt_blocks])
        if it < N_ITER - 1:
            Yn, Zn = Yb[(it + 1) % 2], Zb[(it + 1) % 2]
            mm512(Yn, Y, T, "s")  # Y_next = Y @ T
            mm512(Zn, T, Z, "v")  # Z_next = T @ Z  (T symmetric)
            Y, Z = Yn, Zn
        else:
            # final iteration: only Z needed; C = q * (T @ Z).
            mm512(C, T, Z, "v", scale=rq[:, 1:2])

    # mean replicated blocks: rep_b[p, f] = mean[b*128+p]
    for b in range(4):
        nc.vector.tensor_scalar(
            rep[b][:], ones_bf[:], mean_sb[:, b:b + 1], None, op0=MUL
        )
    ns.release()

    # ------------- wT = C^T @ W^T ; b' = bias - pair-summed w @ mean -------
    wT = wts2.tile([PART, 8, D], bf16, tag="wT")  # w_full^T[i, o]
    for j in range(2):
        WTh = WThs[j]
        for cb in range(4):
            for nb in range(2):
                pt = ps_ns.tile([PART, BLOCK], f32, tag="mm", name="mm")
                for db in range(4):
                    nc.tensor.matmul(
                        pt[:],
                        C[:, db, cb * PART:(cb + 1) * PART],
                        WTh[:, db, nb * BLOCK:(nb + 1) * BLOCK],
                        start=(db == 0),
                        stop=(db == 3),
                    )
                nc.scalar.copy(
                    wT[:, j * 4 + cb, nb * BLOCK:(nb + 1) * BLOCK], pt[:]
                )

    bc_ps = [
        ps_ns.tile([PART, BLOCK], f32, tag="mm", name=f"bc{i}") for i in range(2)
    ]
    for nb in range(2):
        for g in range(8):
            nc.tensor.matmul(
                bc_ps[nb][:],
                rep[g % 4][:],
                wT[:, g, nb * BLOCK:(nb + 1) * BLOCK],
                start=(g == 0),
                stop=(g == 7),
            )
    for nb in range(2):
        nc.vector.tensor_sub(
            b_rep[:, nb * BLOCK:(nb + 1) * BLOCK],
            b_rep[:, nb * BLOCK:(nb + 1) * BLOCK],
            bc_ps[nb][:],
        )

    ps_ns.release()
    wth.release()
    pa.release()
    late.release()

    # ------------- pass D: out = x @ w^T + b' -----------------------------
    ps_D = tc.alloc_tile_pool(name="psumD", bufs=6, space="PSUM", side="left")

    for pi in range(n_row_tiles // 2):
        xT8 = xT_pre[pi] if pi < N_PRE_PAIRS else xT_load(pi)
        for h2 in range(2):
            rt = 2 * pi + h2
            xs = slice(h2 * PART, (h2 + 1) * PART)
            pts = [ps_D.tile([PART, BLOCK], f32, tag="outp", name=f"outp{nb}")
                   for nb in range(2)]
            for g in range(8):
                for nb in range(2):
                    nc.tensor.matmul(
                        pts[nb][:],
                        xT8[:, g, xs],
                        wT[:, g, nb * BLOCK:(nb + 1) * BLOCK],
                        start=(g == 0),
                        stop=(g == 7),
                    )
            ot = pd_out.tile([PART, D], f32, tag="ot", name="ot")
            for nb in range(2):
                nc.vector.tensor_add(
                    ot[:, nb * BLOCK:(nb + 1) * BLOCK], pts[nb][:],
                    b_rep[:, nb * BLOCK:(nb + 1) * BLOCK],
                )
            nc.sync.dma_start(out[rt * PART:(rt + 1) * PART, :], ot[:])

    ps_D.release()
    pd_out.release()
    pd.release()
    stageB2.release()
    wts2.release()
    consts.release()
    dram.release()


# ---------------------------------------------------------------------------
def make_aux_inputs():
    import ml_dtypes

    return {
        "eye15": (1.5 * np.eye(PART)).astype(ml_dtypes.bfloat16),
        "id_bf16": np.eye(PART, dtype=ml_dtypes.bfloat16),
        "id_f32": np.eye(PART, dtype=np.float32),
    }


_NC_CACHE = {}


def get_nc(n_row_tiles=N_ROW_TILES):
    if n_row_tiles not in _NC_CACHE:
        _NC_CACHE[n_row_tiles] = build_nc(n_row_tiles)
    return _NC_CACHE[n_row_tiles]


def make_in_maps(x, weight, bias, n_row_tiles=N_ROW_TILES):
    aux = make_aux_inputs()
    x = np.ascontiguousarray(np.asarray(x, dtype=np.float32))
    weight = np.ascontiguousarray(np.asarray(weight, dtype=np.float32))
    bias = np.asarray(bias, dtype=np.float32)
    bias_rep = np.ascontiguousarray(np.tile(bias[None, :], (PART, 1)))
    rows_pc = n_row_tiles * PART
    in_maps = []
    for i in range(N_CORES):
        m = {"x": x[i * rows_pc:(i + 1) * rows_pc], "weight": weight,
             "bias_rep": bias_rep}
        m.update(aux)
        in_maps.append(m)
    return in_maps


def kernel(x, weight, bias):
    nc = get_nc()
    in_maps = make_in_maps(x, weight, bias)
    res = bass_utils.run_bass_kernel_spmd(
        nc, in_maps, core_ids=list(range(N_CORES))
    )
    return np.concatenate([r["out"] for r in res.results], axis=0)
